# revision 10
# baseline (speedup 1.0000x reference)
"""Causal attention head on 8 TRN2 NeuronCores.

reference: out = softmax(causal((x @ wqk) @ x.T)) @ x @ wov
  x: [4096, 1024] f32, wqk/wov: [1024, 1024] f32.

Sharding: sequence-parallel on query rows with stride-8 interleave -- core m
owns global rows {m, m+8, m+16, ...} (512 rows). This balances the causal
triangle perfectly across cores AND keeps the SPMD graph identical on every
core: the causal mask depends on the core only through its input data
(a host-prepared [128, 1024] additive mask), never through the graph.

Per-core layout: 512 local rows = 4 row tiles of 128 (partition dim).
Local row tile r, local row t' -> global row 1024*r + m + 8*t'.
Row tile r attends to columns [0, 1024*(r+1)): col chunks c = 0..2r+1 of 512.
Chunks c = 2r, 2r+1 are the "diagonal" (mask halves 0/1); earlier chunks are
causally full. Each core runs an identical 20-unit S/PV schedule.

Precision: softmax of std~1024 logits is argmax-sensitive. All operands are
single fp16 (10 mantissa bits): logit error ~0.5 absolute -> sim rel err
1.08e-2 < 2e-2 tol (vs 8.4e-3 for 2-term, 2.4e-3 for 3-term bf16x2). Single
term everywhere = 1/3 the PE work of the bf16x2 baseline.

Tail fix: row tile 3 (the only one whose S spans all 8 chunks) is processed
flash-attention style in two pieces: chunks 0..5 with their own max right
after c=5 (softmax + transposes + 24/32 of its PV hide behind c=6/7 S work),
chunks 6..7 at the end; fin3 rescales and combines the two PSUM accumulators.
This removes ~2/3 of the serial tail after the last S chunk.

DMA: issued on the sync queue in exact consumption order (wqk+xqT -> xt0,1
-> xv cache -> xt2 -> wov -> xt3..7 -> xv tail) so the PE is never starved
waiting behind a prefetch; mask goes first on the gpsimd queue.
"""
import numpy as np

import concourse.bass as bass
import concourse.tile as tile
from concourse import bacc, mybir
from concourse.bass_utils import run_bass_kernel_spmd
from concourse.masks import make_identity

F32 = mybir.dt.float32
F16 = mybir.dt.float16

N = 4096          # sequence length
D = 1024          # model dim
CORES = 8
ROWS = N // CORES  # 512 local rows per core
RT = ROWS // 128   # 4 row tiles
KC = D // 128      # 8 contraction chunks
MASK_VAL = -1e9
XV_CACHE = 24      # xv chunks cached (jc 0..23); jc 24..31 streamed at tail

PRECISION = "fp16"


def _f16(a):
    return np.asarray(a, dtype=np.float16)


def build_nc(precision=PRECISION):
    nc = bacc.Bacc("TRN2", target_bir_lowering=False, debug=False,
                   num_devices=CORES)

    def inp(name, shape, dt):
        return nc.dram_tensor(name, shape, dt, kind="ExternalInput").ap()

    wqk_d = inp("wqk", [D, D], F16)
    xqT = inp("xqT", [D, ROWS], F16)
    xt = inp("xt", [D, N], F16)
    xv_d = inp("xv", [N, D], F16)
    wov_d = inp("wov", [D, D], F16)
    mask_d = inp("mask", [128, 1024], F32)
    out_d = nc.dram_tensor("out", [ROWS, D], F32, kind="ExternalOutput").ap()

    # rearranged views for single-DMA chunked loads: row-block k -> free slot k
    xt_v = xt.rearrange("(k p) j -> p k j", p=128)
    xv_v = xv_d.rearrange("(g p) j -> p g j", p=128)
    wov_v = wov_d.rearrange("(k p) j -> p k j", p=128)

    with tile.TileContext(nc) as tc:
        with (
            tc.tile_pool(name="sb", bufs=1) as sb,
            tc.tile_pool(name="sb2", bufs=2) as sb2,
            tc.tile_pool(name="ps_mm", bufs=2, space="PSUM") as ps_mm,
            tc.tile_pool(name="ps_att", bufs=4, space="PSUM") as ps_att,
            tc.tile_pool(name="ps_tp", bufs=2, space="PSUM") as ps_tp,
        ):
            ident = sb.tile([128, 128], F16, tag="ident")
            make_identity(nc, ident[:])
            mask_sb = sb.tile([128, 1024], F32, tag="mask")
            nc.gpsimd.dma_start(mask_sb[:], mask_d[:])

            # ---- Q-side inputs: 4 coarse DMAs (per-dma_start queue
            # overhead is ~1.3us; 16 small chunk loads paced the Q phase) ----
            wqk_sb = sb.tile([128, KC * D], F16, tag="wqk_s")
            xqT_sb = sb.tile([128, KC * ROWS], F16, tag="xq")
            wqk_v = wqk_d.rearrange("(k p) j -> p k j", p=128)
            xqT_v = xqT.rearrange("(k p) t -> p k t", p=128)
            for h in range(2):
                nc.sync.dma_start(
                    xqT_sb[:, h * 4 * ROWS: (h + 1) * 4 * ROWS].rearrange(
                        "p (k t) -> p k t", k=4),
                    xqT_v[:, h * 4: (h + 1) * 4, :])
                nc.sync.dma_start(
                    wqk_sb[:, h * 4 * D: (h + 1) * 4 * D].rearrange(
                        "p (k j) -> p k j", k=4),
                    wqk_v[:, h * 4: (h + 1) * 4, :])

            # ---- HAM warm-up: ~6us of dummy matmuls while the first DMAs
            # are in flight, so real matmuls start at the warm 2.4GHz clock
            # instead of 1.2GHz (activity window is ~3.4us) ----
            warm_ps = ps_tp.tile([128, 128], F32, tag="tp", name="warm")
            for _ in range(96):
                nc.tensor.matmul(warm_ps[:], ident[:], ident[:],
                                 start=True, stop=True)

            # ---- Q phase: QT[d', t] chunks, single fp16 term ----
            qt = sb.tile([128, KC * ROWS], F16, tag="qt")
            for q in range(KC):
                acc = ps_mm.tile([128, ROWS], F32, tag="mm")
                for k in range(KC):
                    nc.tensor.matmul(
                        acc[:],
                        wqk_sb[:, k * D + q * 128: k * D + (q + 1) * 128],
                        xqT_sb[:, bass.ts(k, ROWS)],
                        start=(k == 0), stop=(k == KC - 1))
                nc.vector.tensor_copy(qt[:, bass.ts(q, ROWS)], acc[:])

            # ---- persistent per-row-tile buffers ----
            s_off = [0, 1024, 3072, 6144]
            s_len = [(2 * r + 2) * 512 for r in range(RT)]
            s_all = sb.tile([128, 10240], F32, tag="wqk_s")
            pt_all = sb.tile([128, 10240], F16, tag="pt")
            xv_cache = sb.tile([128, XV_CACHE * D], F16, tag="xvc")

            mx_all = sb.tile([128, 8 * RT], F32, tag="mx")
            negmax = sb.tile([128, RT], F32, tag="negmax")
            lsum = sb.tile([128, RT], F32, tag="lsum")
            lpart = sb.tile([128, 2 * RT], F32, tag="lpart")
            linv = sb.tile([128, RT], F32, tag="linv")
            # tile-3 flash-split stats: [pm_a, pm_b, nm_a, nm_b, sa0, sa1,
            #                            sum_a, sum_b, nm3, f_a, f_b,
            #                            sfa, sfb, l3, linv3_ca_cb...]
            st3 = sb.tile([128, 16], F32, tag="st3")

            # gpsimd queue: mask (above) then wov + xv tail -- all small and
            # early; the big sync-queue stream below never waits behind them.
            wov_sb = sb.tile([128, KC * D], F16, tag="wov")
            nc.gpsimd.dma_start(
                wov_sb[:].rearrange("p (k j) -> p k j", k=KC), wov_v[:])
            xv_tail = sb.tile([128, 8 * D], F16, tag="xvt")
            nc.gpsimd.dma_start(
                xv_tail[:].rearrange("p (g j) -> p g j", g=8),
                xv_v[:, XV_CACHE: XV_CACHE + 8, :])

            xt_cur = {}

            def dma_xt(c):
                xt_sb = sb2.tile([128, KC * 512], F16, tag="xt",
                                 name=f"xt_c{c}")
                nc.sync.dma_start(
                    xt_sb[:].rearrange("p (k j) -> p k j", k=KC),
                    xt_v[:, :, bass.ts(c, 512)])
                xt_cur["t"] = xt_sb

            def s_chunk(r, c):
                acc = ps_mm.tile([128, 512], F32, tag="mm",
                                 name=f"s_r{r}c{c}")
                for k in range(KC):
                    nc.tensor.matmul(
                        acc[:],
                        qt[:, k * ROWS + r * 128: k * ROWS + (r + 1) * 128],
                        xt_cur["t"][:, bass.ts(k, 512)],
                        start=(k == 0), stop=(k == KC - 1))
                dst = s_all[:, s_off[r] + c * 512: s_off[r] + (c + 1) * 512]
                if c == 2 * r:
                    nc.vector.tensor_add(dst, acc[:], mask_sb[:, 0:512])
                elif c == 2 * r + 1:
                    nc.vector.tensor_add(dst, acc[:], mask_sb[:, 512:1024])
                else:
                    nc.scalar.copy(dst, acc[:])
                # per-chunk row max (pipelines the softmax stats)
                nc.vector.tensor_reduce(
                    out=mx_all[:, r * 8 + c: r * 8 + c + 1], in_=dst,
                    op=mybir.AluOpType.max, axis=mybir.AxisListType.X)

            p_tiles = {}
            att_tiles = {}

            def stats(r):
                """combine chunk maxes -> exp -> row sums (DVE/ACT only)"""
                nm = negmax[:, r: r + 1]
                nc.vector.tensor_reduce(
                    out=nm, in_=mx_all[:, r * 8: r * 8 + 2 * r + 2],
                    op=mybir.AluOpType.max, axis=mybir.AxisListType.X,
                    negate=True)
                p_r = sb2.tile([128, s_len[RT - 1]], F16, tag="p_r",
                               name=f"p_r{r}")
                half = (s_len[r] // 2 + 511) // 512 * 512 if s_len[r] > 2048 else s_len[r]
                pieces = [(0, half)]
                if half < s_len[r]:
                    pieces.append((half, s_len[r] - half))
                for pi, (off, ln) in enumerate(pieces):
                    nc.scalar.activation(
                        p_r[:, off: off + ln],
                        s_all[:, s_off[r] + off: s_off[r] + off + ln],
                        mybir.ActivationFunctionType.Exp,
                        bias=nm, scale=1.0,
                        accum_out=lpart[:, 2 * r + pi: 2 * r + pi + 1])
                if len(pieces) == 1:
                    nc.vector.reciprocal(linv[:, r: r + 1],
                                         lpart[:, 2 * r: 2 * r + 1])
                else:
                    nc.vector.tensor_add(lsum[:, r: r + 1],
                                         lpart[:, 2 * r: 2 * r + 1],
                                         lpart[:, 2 * r + 1: 2 * r + 2])
                    nc.vector.reciprocal(linv[:, r: r + 1], lsum[:, r: r + 1])
                p_tiles[r] = p_r

            def p_transpose(r, jc0, jc1):
                """transpose P[:, jc0*128:jc1*128] into pt_all (PE)"""
                p_r = p_tiles[r]
                for g in range((jc1 - jc0) // 4):
                    pt_ps = ps_tp.tile([128, 512], F16, tag="tp",
                                       name=f"pt_r{r}g{jc0}_{g}")
                    for i in range(4):
                        jc = jc0 + g * 4 + i
                        nc.tensor.matmul(
                            pt_ps[:, bass.ts(i, 128)],
                            p_r[:, bass.ts(jc, 128)],
                            ident[:], is_transpose=True,
                            start=(i == 0), stop=(i == 3))
                    nc.vector.tensor_copy(
                        pt_all[:, s_off[r] + (jc0 + g * 4) * 128:
                               s_off[r] + (jc0 + (g + 1) * 4) * 128],
                        pt_ps[:])

            def pv(r, att_ps, jc0, jc1, first, last):
                """PV matmuls for key chunks [jc0, jc1) into att_ps pair"""
                for jc in range(jc0, jc1):
                    if jc < XV_CACHE:
                        xv_t, j2 = xv_cache, jc
                    else:
                        xv_t, j2 = xv_tail, jc - XV_CACHE
                    lhs = pt_all[:, s_off[r] + jc * 128:
                                 s_off[r] + (jc + 1) * 128]
                    for h in range(2):
                        nc.tensor.matmul(
                            att_ps[h][:], lhs,
                            xv_t[:, j2 * D + h * 512: j2 * D + (h + 1) * 512],
                            start=(first and jc == jc0),
                            stop=(last and jc == jc1 - 1))

            def tpv(r):
                p_transpose(r, 0, s_len[r] // 128)
                att_ps = [ps_att.tile([128, 512], F32, tag="att",
                                      name=f"att_r{r}h{h}")
                          for h in range(2)]
                att_tiles[r] = att_ps
                pv(r, att_ps, 0, 8 * (r + 1), True, True)

            def fin(r):
                """att normalize + transpose + OV + output DMA"""
                att_ps = att_tiles[r]
                att_sb = sb2.tile([128, D], F16, tag="att_sb", bufs=1,
                                  name=f"att_sb{r}")
                for h in range(2):
                    nc.scalar.mul(att_sb[:, bass.ts(h, 512)], att_ps[h][:],
                                  linv[:, r: r + 1])
                fin_tail(r, att_sb)

            def fin_tail(r, att_sb):
                attT = sb2.tile([128, D], F16, tag="attT", bufs=1,
                                name=f"attT{r}")
                for g in range(2):
                    at_ps = ps_tp.tile([128, 512], F16, tag="tp",
                                       name=f"at_r{r}g{g}")
                    for i in range(4):
                        nc.tensor.matmul(at_ps[:, bass.ts(i, 128)],
                                         att_sb[:, bass.ts(g * 4 + i, 128)],
                                         ident[:], is_transpose=True,
                                         start=(i == 0), stop=(i == 3))
                    nc.vector.tensor_copy(attT[:, bass.ts(g, 512)], at_ps[:])
                out_sb = sb2.tile([128, D], F32, tag="out_sb", bufs=1,
                                  name=f"out_sb{r}")
                for h in range(2):
                    acc = ps_mm.tile([128, 512], F32, tag="mm",
                                     name=f"ov_r{r}h{h}")
                    for k in range(KC):
                        nc.tensor.matmul(
                            acc[:], attT[:, bass.ts(k, 128)],
                            wov_sb[:, k * D + h * 512: k * D + (h + 1) * 512],
                            start=(k == 0), stop=(k == KC - 1))
                    nc.scalar.copy(out_sb[:, bass.ts(h, 512)], acc[:])
                nc.gpsimd.dma_start(out_d[bass.ts(r, 128), :], out_sb[:])

            # ---- tile-3 flash split: A = chunks 0..6, B = chunk 7 ----
            def stats3a():
                nc.vector.tensor_reduce(
                    out=st3[:, 2:3], in_=mx_all[:, 24:31],
                    op=mybir.AluOpType.max, axis=mybir.AxisListType.X,
                    negate=True)
                nc.vector.tensor_reduce(
                    out=st3[:, 0:1], in_=mx_all[:, 24:31],
                    op=mybir.AluOpType.max, axis=mybir.AxisListType.X)
                p_r = sb2.tile([128, s_len[3]], F16, tag="p_r", name="p_r3")
                p_tiles[3] = p_r
                for pi, (off, ln) in enumerate([(0, 2048), (2048, 1536)]):
                    nc.scalar.activation(
                        p_r[:, off: off + ln],
                        s_all[:, s_off[3] + off: s_off[3] + off + ln],
                        mybir.ActivationFunctionType.Exp,
                        bias=st3[:, 2:3], scale=1.0,
                        accum_out=st3[:, 4 + pi: 5 + pi])
                nc.vector.tensor_add(st3[:, 6:7], st3[:, 4:5], st3[:, 5:6])

            def stats3b():
                nc.vector.tensor_reduce(
                    out=st3[:, 3:4], in_=mx_all[:, 31:32],
                    op=mybir.AluOpType.max, axis=mybir.AxisListType.X,
                    negate=True)
                nc.vector.tensor_reduce(
                    out=st3[:, 1:2], in_=mx_all[:, 31:32],
                    op=mybir.AluOpType.max, axis=mybir.AxisListType.X)
                p_r = p_tiles[3]
                nc.scalar.activation(
                    p_r[:, 3584:4096],
                    s_all[:, s_off[3] + 3584: s_off[3] + 4096],
                    mybir.ActivationFunctionType.Exp,
                    bias=st3[:, 3:4], scale=1.0,
                    accum_out=st3[:, 7:8])
                # the whole combine-coefficient chain only needs sums+maxes,
                # so it runs here, hidden behind fin(2)/tail PE work:
                # nm3 = -max(m_a, m_b); f_x = exp(pm_x + nm3)
                nc.vector.tensor_reduce(
                    out=st3[:, 8:9], in_=st3[:, 0:2],
                    op=mybir.AluOpType.max, axis=mybir.AxisListType.X,
                    negate=True)
                for i in range(2):
                    nc.scalar.activation(
                        st3[:, 9 + i: 10 + i], st3[:, i: i + 1],
                        mybir.ActivationFunctionType.Exp,
                        bias=st3[:, 8:9], scale=1.0)
                # l3 = sum_a*f_a + sum_b*f_b ; ca/cb = f_x / l3
                nc.vector.tensor_mul(st3[:, 11:12], st3[:, 6:7], st3[:, 9:10])
                nc.vector.tensor_mul(st3[:, 12:13], st3[:, 7:8], st3[:, 10:11])
                nc.vector.tensor_add(st3[:, 13:14], st3[:, 11:12],
                                     st3[:, 12:13])
                nc.vector.reciprocal(st3[:, 14:15], st3[:, 13:14])
                nc.vector.tensor_mul(st3[:, 9:10], st3[:, 9:10],
                                     st3[:, 14:15])
                nc.vector.tensor_mul(st3[:, 10:11], st3[:, 10:11],
                                     st3[:, 14:15])

            def fin3():
                att_a, att_b = att_tiles["3a"], att_tiles["3b"]
                att_sb = sb2.tile([128, D], F16, tag="att_sb", bufs=1,
                                  name="att_sb3")
                t_b = sb2.tile([128, D], F16, tag="t_b", bufs=1, name="t_b3")
                for h in range(2):
                    nc.scalar.mul(att_sb[:, bass.ts(h, 512)], att_a[h][:],
                                  st3[:, 9:10])
                    nc.scalar.mul(t_b[:, bass.ts(h, 512)], att_b[h][:],
                                  st3[:, 10:11])
                nc.vector.tensor_add(att_sb[:], att_sb[:], t_b[:])
                fin_tail(3, att_sb)

            # ---------------- schedule ----------------
            for c in range(2 * RT):
                dma_xt(c)
                if c in (1, 2, 3):
                    # xv cache thirds, interleaved between xt chunks on the
                    # sync queue: piece i covers jc [8*(c-1), 8*c), needed by
                    # tpv(c-1) which runs ~2 chunks later.
                    g0 = 8 * (c - 1)
                    nc.sync.dma_start(
                        xv_cache[:, g0 * D: (g0 + 8) * D].rearrange(
                            "p (g j) -> p g j", g=8),
                        xv_v[:, g0: g0 + 8, :])
                for r in range(c // 2, RT):
                    s_chunk(r, c)
                if c >= 2 and c % 2 == 0:
                    tpv(c // 2 - 1)
                if c % 2 == 1 and c < 7:
                    stats((c - 1) // 2)
                if c == 6:
                    stats3a()
                if c == 7:
                    stats3b()
                if c >= 3 and c % 2 == 1:
                    fin(c // 2 - 1)
                if c == 7:
                    # transposes + 28/32 of tile-3 PV hide behind fin(2)
                    p_transpose(3, 0, 28)
                    att_a = [ps_att.tile([128, 512], F32, tag="att",
                                         name=f"att_3ah{h}")
                             for h in range(2)]
                    att_tiles["3a"] = att_a
                    pv(3, att_a, 0, 28, True, True)
            # ---- tail: only chunk 7 of tile 3 remains ----
            p_transpose(3, 28, 32)
            att_b = [ps_att.tile([128, 512], F32, tag="att",
                                 name=f"att_3bh{h}")
                     for h in range(2)]
            att_tiles["3b"] = att_b
            pv(3, att_b, 28, 32, True, True)
            fin3()

    nc.compile()
    return nc


_NC_CACHE = {}


def _get_nc(precision=PRECISION):
    if precision not in _NC_CACHE:
        _NC_CACHE[precision] = build_nc(precision)
    return _NC_CACHE[precision]


def make_in_maps(x, wqk, wov):
    x = np.ascontiguousarray(x, dtype=np.float32)
    wqk = np.ascontiguousarray(wqk, dtype=np.float32)
    wov = np.ascontiguousarray(wov, dtype=np.float32)

    xt = np.ascontiguousarray(x.T)
    shared = {"xv": _f16(x), "wov": _f16(wov), "wqk": _f16(wqk),
              "xt": _f16(xt)}

    in_maps = []
    t_idx = np.arange(128)
    c_idx = np.arange(1024)
    for m in range(CORES):
        xqT = np.ascontiguousarray(x[m::CORES].T)
        mask = np.where(c_idx[None, :] <= m + 8 * t_idx[:, None],
                        0.0, MASK_VAL).astype(np.float32)
        im = dict(shared)
        im.update({"mask": mask, "xqT": _f16(xqT)})
        in_maps.append(im)
    return in_maps


def kernel(x, wqk, wov, precision=PRECISION, _trace=False):
    nc = _get_nc()
    in_maps = make_in_maps(x, wqk, wov)
    res = run_bass_kernel_spmd(nc, in_maps, core_ids=list(range(CORES)),
                               trace=_trace)
    out = np.empty((N, D), dtype=np.float32)
    for m in range(CORES):
        out[m::CORES] = res.results[m]["out"]
    if _trace:
        kernel.last_results = res
    return out
